# revision 21
# baseline (speedup 1.0000x reference)
"""GCN layer (SpMM) Bass kernel for 8 trn2 NeuronCores.

out[i] = sum_{e: rows[e]==i} edge_vals[e] * embeds[cols[e]]
N=100000 nodes, E=1000000 edges, D=64 features.

Strategy: host sorts edges by destination row and splits nodes into 8
contiguous ranges (12500 nodes/core) with disjoint outputs -> no
collectives. Per core, output rows are processed in blocks of 128; each
block's edges are padded to chunks of 128 edges. Work is batched per
gather instruction (G_CHUNKS chunks = G_CHUNKS*128 edges):
  1. SWDGE dma_gather     gt[p,c,:]   = embeds[cols[p,c], :]   (gpsimd)
  2. scale+cast (bf16)    emb[p,c,:]  = gt[p,c,:] * vals[p,c]  (vector)
  3. one-hot (bf16)       oh[p,c,r]   = (rrow[p,c] == r)       (vector)
  4. per chunk c: matmul  psum[r,:]  += oh[:,c,:].T @ emb[:,c,:]  (tensor)
After a block's chunks, PSUM is copied to SBUF and DMA'd to the output
rows (contiguous -> plain DMA, no scatter).

The gather uses SWDGE dma_gather (InstDMAGatherAnt): one instruction
fetches up to 1024 rows of 256B spread over all 16 DMA engines (the
SWDGE descriptor ring holds 1024 descriptors -> num_idxs <= 1024),
instead of one HWDGE indirect DMA per chunk serialized on the single
qPoolDynamic queue. dma_gather indices are int16, so the embeds table is
split into 4 views of <=25000 rows; each block's edges are grouped by
col-quartile on the host and chunk-padded per group. Chunk slots are
laid out group-major so each group's gathers cover long consecutive
token runs.

The chunk schedule is computed from the data on the host and baked into
the program; all 8 cores share one program, so per-(block,group) chunk
counts are the max over cores.
"""

import os
import sys

import numpy as np

if "/opt/trn_rl_repo" not in sys.path:
    sys.path.insert(0, "/opt/trn_rl_repo")

N_NODES = 100000
D = 64
P = 128
N_CORES = 8
N_GROUPS = 4  # embeds views (int16 gather idx => <=32768 rows per view)
# chunks per dma_gather instruction; SWDGE ring holds 1024 descriptors
# (dynamic_dma_scratch_size 16384 / 16B) and one instruction's descriptors
# must fit: num_idxs = G_CHUNKS*128 <= 1024 (G=16 wedges the device).
G_CHUNKS = int(os.environ.get("G_CHUNKS", "8"))
GAT_BUFS = int(os.environ.get("GAT_BUFS", "3"))


def _schedule(chunks_bg):
    """Group-major slot layout. Returns (Tg, gbase, n_chunks, off_bg)."""
    n_blocks = chunks_bg.shape[0]
    Tg = chunks_bg.sum(axis=0).astype(np.int64)  # chunks per group
    gbase = np.concatenate([[0], np.cumsum(Tg)]).astype(np.int64)
    off_bg = np.zeros((n_blocks, N_GROUPS), np.int64)
    for g in range(N_GROUPS):
        off_bg[:, g] = gbase[g] + np.concatenate(
            [[0], np.cumsum(chunks_bg[:-1, g])]
        )
    return Tg, gbase, int(chunks_bg.sum()), off_bg


def _build_program(chunks_bg, n_nodes):
    import concourse.bacc as bacc
    import concourse.tile as tile
    from concourse import mybir

    n_blocks = chunks_bg.shape[0]
    group_size = -(-n_nodes // N_GROUPS)
    Tg, gbase, n_chunks, off_bg = _schedule(chunks_bg)

    nc = bacc.Bacc(
        "TRN2",
        target_bir_lowering=False,
        debug=False,
        num_devices=N_CORES,
        num_swdge_queues=4,
        dynamic_dma_scratch_size=int(os.environ.get("DMA_SCRATCH", "16384")),
    )
    embeds_t = nc.dram_tensor(
        "embeds", [n_nodes, D], mybir.dt.float32, kind="ExternalInput"
    )
    idx_t = nc.dram_tensor(
        "idx_p", [P, n_chunks * (P // 16)], mybir.dt.int16, kind="ExternalInput"
    )
    vals_t = nc.dram_tensor("vals_p", [P, n_chunks], mybir.dt.float32, kind="ExternalInput")
    rrow_t = nc.dram_tensor("rrow_p", [P, n_chunks], mybir.dt.bfloat16, kind="ExternalInput")
    # iota[p, r, c] = r  (row index along middle dim, repeated per chunk c)
    iota_t = nc.dram_tensor("iota", [P, P, G_CHUNKS], mybir.dt.bfloat16, kind="ExternalInput")
    out_t = nc.dram_tensor(
        "out", [n_blocks * P, D], mybir.dt.bfloat16, kind="ExternalOutput"
    )

    with tile.TileContext(nc) as tc:
        with (
            tc.tile_pool(name="static", bufs=1) as static_pool,
            tc.tile_pool(name="gat0", bufs=GAT_BUFS) as gp0,
            tc.tile_pool(name="gat1", bufs=GAT_BUFS) as gp1,
            tc.tile_pool(name="gat2", bufs=GAT_BUFS) as gp2,
            tc.tile_pool(name="gat3", bufs=GAT_BUFS) as gp3,
            tc.tile_pool(name="emb0", bufs=GAT_BUFS) as ep0,
            tc.tile_pool(name="emb1", bufs=GAT_BUFS) as ep1,
            tc.tile_pool(name="emb2", bufs=GAT_BUFS) as ep2,
            tc.tile_pool(name="emb3", bufs=GAT_BUFS) as ep3,
            tc.tile_pool(name="ohv0", bufs=GAT_BUFS) as op0,
            tc.tile_pool(name="ohv1", bufs=GAT_BUFS) as op1,
            tc.tile_pool(name="ohv2", bufs=GAT_BUFS) as op2,
            tc.tile_pool(name="ohv3", bufs=GAT_BUFS) as op3,
            tc.tile_pool(name="outp", bufs=4) as out_pool,
            tc.tile_pool(name="psum", bufs=4, space="PSUM") as psum_pool,
        ):
            idx_sb = static_pool.tile([P, n_chunks * (P // 16)], mybir.dt.int16)
            vals_sb = static_pool.tile([P, n_chunks], mybir.dt.float32)
            rrow_sb = static_pool.tile([P, n_chunks], mybir.dt.bfloat16)
            iota_sb = static_pool.tile([P, P, G_CHUNKS], mybir.dt.bfloat16)
            nc.sync.dma_start(out=idx_sb[:], in_=idx_t[:])
            nc.sync.dma_start(out=vals_sb[:], in_=vals_t[:])
            nc.sync.dma_start(out=rrow_sb[:], in_=rrow_t[:])
            nc.sync.dma_start(out=iota_sb[:], in_=iota_t[:])

            gat_pools = [gp0, gp1, gp2, gp3]
            emb_pools = [ep0, ep1, ep2, ep3]
            ohv_pools = [op0, op1, op2, op3]
            # per group: list of (emb_tile, ohv_tile) per gather batch
            btiles = [[] for _ in range(N_GROUPS)]
            next_instr = [0] * N_GROUPS
            qrr = [0]  # SWDGE queue round-robin
            prev_gather = [None]  # nosync chain so scheduled order == emission
            # order (tile's DMASW sem lanes rotate mod 8 in scheduled order;
            # each lane is locked to one SWDGE queue, so queue rotation must
            # stay aligned with lane rotation)

            def ensure(g, upto_gc):
                # emit batches for group g until group-chunks [0, upto_gc) covered
                while next_instr[g] * G_CHUNKS < upto_gc:
                    j = next_instr[g]
                    g0 = j * G_CHUNKS
                    n_i = int(min(G_CHUNKS, Tg[g] - g0))
                    slot0 = int(gbase[g] + g0)
                    gt = gat_pools[g].tile([P, G_CHUNKS, D], mybir.dt.float32)
                    r0 = g * group_size
                    r1 = min((g + 1) * group_size, n_nodes)
                    gi = nc.gpsimd.dma_gather(
                        gt[:, :n_i, :],
                        embeds_t[r0:r1, :],
                        idx_sb[:, slot0 * (P // 16) : (slot0 + n_i) * (P // 16)],
                        n_i * P,
                        n_i * P,
                        D,
                        queue_num=qrr[0],
                    )
                    if prev_gather[0] is not None:
                        from concourse.instruction_name_ordered_set import (
                            InstructionNameOrderedSet,
                        )

                        dep = InstructionNameOrderedSet()
                        dep.add(prev_gather[0])
                        gi.ins.add_nosync_dependencies_from(dep)
                    prev_gather[0] = gi.ins.name
                    qrr[0] = (qrr[0] + 1) % 4
                    emb = emb_pools[g].tile([P, G_CHUNKS, D], mybir.dt.bfloat16)
                    nc.vector.tensor_tensor(
                        out=emb[:, :n_i, :],
                        in0=gt[:, :n_i, :],
                        in1=vals_sb[:, slot0 : slot0 + n_i].to_broadcast(
                            [P, n_i, D]
                        ),
                        op=mybir.AluOpType.mult,
                    )
                    # one-hot in [P, r, c] layout: inner (chunk) dim is packed
                    # in all operands -> DVE 2x_1p mode applies
                    ohv = ohv_pools[g].tile([P, P, G_CHUNKS], mybir.dt.bfloat16)
                    nc.vector.tensor_tensor(
                        out=ohv[:, :, :n_i],
                        in0=rrow_sb[:, slot0 : slot0 + n_i]
                        .to_broadcast([P, n_i, P])
                        .transpose([0, 2, 1]),
                        in1=iota_sb[:, :, :n_i],
                        op=mybir.AluOpType.is_equal,
                    )
                    btiles[g].append((emb, ohv))
                    next_instr[g] += 1

            for b in range(n_blocks):
                tot_b = int(chunks_bg[b].sum())
                psum_tile = psum_pool.tile([P, D], dtype=mybir.dt.float32, space="PSUM")
                t = 0
                for g in range(N_GROUPS):
                    cbg = int(chunks_bg[b, g])
                    if cbg == 0:
                        continue
                    gc0 = int(off_bg[b, g] - gbase[g])
                    ensure(g, gc0 + cbg)
                    for c in range(cbg):
                        gc = gc0 + c
                        emb, ohv = btiles[g][gc // G_CHUNKS]
                        o = gc % G_CHUNKS
                        nc.tensor.matmul(
                            out=psum_tile[:],
                            lhsT=ohv[:, :, o],
                            rhs=emb[:, o, :],
                            start=(t == 0),
                            stop=(t == tot_b - 1),
                        )
                        t += 1
                o_sb = out_pool.tile([P, D], mybir.dt.bfloat16)
                nc.scalar.copy(out=o_sb[:], in_=psum_tile[:])
                nc.sync.dma_start(out=out_t[b * P : (b + 1) * P, :], in_=o_sb[:])
    nc.compile()
    return nc


def _kernel_impl(rows, cols, edge_vals, embeds, n_nodes, trace=False):
    import ml_dtypes

    from concourse.bass_utils import run_bass_kernel_spmd

    rows = np.asarray(rows).astype(np.int64)
    cs_all = np.asarray(cols).astype(np.int32)
    vs_all = np.asarray(edge_vals).astype(np.float32)
    embeds = np.ascontiguousarray(np.asarray(embeds), dtype=np.float32)

    npc = n_nodes // N_CORES
    assert npc * N_CORES == n_nodes
    n_blocks = (npc + P - 1) // P
    group_size = -(-n_nodes // N_GROUPS)
    assert group_size <= 32767

    core = rows // npc
    blk = (rows % npc) // P
    rrow = (rows % npc) % P
    grp = cs_all // group_size
    bkey = ((core * n_blocks + blk) * N_GROUPS + grp).astype(np.int64)
    order = np.argsort(bkey, kind="stable")
    bkey_s = bkey[order]
    cs_s = cs_all[order]
    vs_s = vs_all[order]
    rrow_s = rrow[order]

    n_seg = N_CORES * n_blocks * N_GROUPS
    cnt = np.bincount(bkey_s, minlength=n_seg).reshape(N_CORES, n_blocks, N_GROUPS)
    chunks_bg = -(-cnt.max(axis=0) // P)  # [n_blocks, N_GROUPS]
    forced = chunks_bg.sum(axis=1) == 0
    chunks_bg[forced, 0] = 1
    Tg, gbase, n_chunks, off_bg = _schedule(chunks_bg)

    # position of each edge inside its (core, block, group) segment
    seg_start = np.zeros(n_seg + 1, np.int64)
    np.cumsum(cnt.ravel(), out=seg_start[1:])
    pos_in_seg = np.arange(len(rows), dtype=np.int64) - seg_start[bkey_s]

    b_s = (bkey_s // N_GROUPS) % n_blocks
    g_s = bkey_s % N_GROUPS
    k_s = bkey_s // (n_blocks * N_GROUPS)
    slot_s = off_bg[b_s, g_s] + pos_in_seg // P  # global chunk slot
    part_s = pos_in_seg % P

    vals_p = np.zeros((N_CORES, n_chunks, P), np.float32)
    rrow_p = np.zeros((N_CORES, n_chunks, P), np.float32)
    tok_p = np.zeros((N_CORES, n_chunks, P), np.int16)
    vals_p[k_s, slot_s, part_s] = vs_s
    rrow_p[k_s, slot_s, part_s] = rrow_s
    tok_p[k_s, slot_s, part_s] = (cs_s - g_s * group_size).astype(np.int16)

    # device layouts
    vals_d = np.ascontiguousarray(vals_p.transpose(0, 2, 1))  # [8, P, n_chunks]
    rrow_d = np.ascontiguousarray(rrow_p.transpose(0, 2, 1)).astype(ml_dtypes.bfloat16)
    # idx wrap: token t at partition t%16, col t//16; replicate to 128 partitions
    idx_d = np.ascontiguousarray(
        np.tile(
            tok_p.reshape(N_CORES, n_chunks * P // 16, 16).transpose(0, 2, 1),
            (1, 8, 1),
        )
    )  # [8, 128, n_chunks*8]
    # iota[p, r, c] = r
    iota = np.ascontiguousarray(
        np.broadcast_to(
            np.arange(P, dtype=np.float32)[None, :, None], (P, P, G_CHUNKS)
        )
    ).astype(ml_dtypes.bfloat16)

    nc = _build_program(chunks_bg, n_nodes)
    in_maps = [
        {
            "embeds": embeds,
            "idx_p": idx_d[k],
            "vals_p": vals_d[k],
            "rrow_p": rrow_d[k],
            "iota": iota,
        }
        for k in range(N_CORES)
    ]
    global _LAST
    _LAST = (nc, in_maps)
    r = run_bass_kernel_spmd(nc, in_maps, list(range(N_CORES)), trace=trace)
    out = np.concatenate(
        [r.results[k]["out"][:npc] for k in range(N_CORES)], axis=0
    ).astype(np.float32)
    if trace:
        return out, r
    return out


_LAST = None


def kernel(rows, cols, edge_vals, embeds):
    return _kernel_impl(rows, cols, edge_vals, embeds, N_NODES)


# revision 25
# speedup vs baseline: 1.0404x; 1.0404x over previous
"""GCN layer (SpMM) Bass kernel for 8 trn2 NeuronCores.

out[i] = sum_{e: rows[e]==i} edge_vals[e] * embeds[cols[e]]
N=100000 nodes, E=1000000 edges, D=64 features.

Strategy: host sorts edges by destination row and splits nodes into 8
contiguous ranges (12500 nodes/core) with disjoint outputs -> no
collectives. Per core, output rows are processed in blocks of 128; each
block's edges are padded to chunks of 128 edges. Work is batched per
gather instruction (G_CHUNKS chunks = G_CHUNKS*128 edges):
  1. SWDGE dma_gather     gt[p,c,:]   = embeds[cols[p,c], :]   (gpsimd)
  2. scale+cast (bf16)    emb[p,c,:]  = gt[p,c,:] * vals[p,c]  (vector)
  3. one-hot (bf16)       oh[p,r,c]   = (rrow[p,c] == r)       (vector,
     [p,r,c] layout keeps the packed chunk dim innermost in every
     operand -> DVE 2x_1p mode)
  4. per chunk c: matmul  psum[r,:]  += oh[:,:,c].T @ emb[:,c,:]  (tensor)
After a block's chunks, PSUM is copied to SBUF and DMA'd to the output
rows (contiguous -> plain DMA, no scatter).

The gather uses SWDGE dma_gather (InstDMAGatherAnt): one instruction
fetches up to 1024 rows of 256B spread over all 16 DMA engines (the
SWDGE descriptor ring holds 1024 descriptors -> num_idxs <= 1024),
instead of one HWDGE indirect DMA per chunk serialized on the single
qPoolDynamic queue. dma_gather indices are int16, so the embeds table is
split into 4 views of <=25000 rows; each block's edges are grouped by
col-quartile on the host and chunk-padded per group. Chunk slots are
laid out group-major so each group's gathers cover long consecutive
token runs.

The chunk schedule is computed from the data on the host and baked into
the program; all 8 cores share one program, so per-(block,group) chunk
counts are the max over cores.
"""

import os
import sys

import numpy as np

if "/opt/trn_rl_repo" not in sys.path:
    sys.path.insert(0, "/opt/trn_rl_repo")

N_NODES = 100000
D = 64
P = 128
N_CORES = 8
N_GROUPS = 4  # embeds views (int16 gather idx => <=32768 rows per view)
# chunks per dma_gather instruction; SWDGE ring holds 1024 descriptors
# (dynamic_dma_scratch_size 16384 / 16B) and one instruction's descriptors
# must fit: num_idxs = G_CHUNKS*128 <= 1024 (G=16 wedges the device).
G_CHUNKS = int(os.environ.get("G_CHUNKS", "8"))
GAT_BUFS = int(os.environ.get("GAT_BUFS", "3"))


def _schedule(chunks_bg):
    """Group-major slot layout. Returns (Tg, gbase, n_chunks, off_bg)."""
    n_blocks = chunks_bg.shape[0]
    Tg = chunks_bg.sum(axis=0).astype(np.int64)  # chunks per group
    gbase = np.concatenate([[0], np.cumsum(Tg)]).astype(np.int64)
    off_bg = np.zeros((n_blocks, N_GROUPS), np.int64)
    for g in range(N_GROUPS):
        off_bg[:, g] = gbase[g] + np.concatenate(
            [[0], np.cumsum(chunks_bg[:-1, g])]
        )
    return Tg, gbase, int(chunks_bg.sum()), off_bg


def _build_program(chunks_bg, n_nodes):
    import concourse.bacc as bacc
    import concourse.tile as tile
    from concourse import mybir

    n_blocks = chunks_bg.shape[0]
    group_size = -(-n_nodes // N_GROUPS)
    Tg, gbase, n_chunks, off_bg = _schedule(chunks_bg)

    nc = bacc.Bacc(
        "TRN2",
        target_bir_lowering=False,
        debug=False,
        num_devices=N_CORES,
        num_swdge_queues=4,
        dynamic_dma_scratch_size=int(os.environ.get("DMA_SCRATCH", "16384")),
    )
    embeds_t = nc.dram_tensor(
        "embeds", [n_nodes, D], mybir.dt.float32, kind="ExternalInput"
    )
    idx_t = nc.dram_tensor(
        "idx_p", [P, n_chunks * (P // 16)], mybir.dt.int16, kind="ExternalInput"
    )
    vals_t = nc.dram_tensor("vals_p", [P, n_chunks], mybir.dt.float32, kind="ExternalInput")
    rrow_t = nc.dram_tensor("rrow_p", [P, n_chunks], mybir.dt.bfloat16, kind="ExternalInput")
    # iota[p, r, c] = r  (row index along middle dim, repeated per chunk c)
    iota_t = nc.dram_tensor("iota", [P, P, G_CHUNKS], mybir.dt.bfloat16, kind="ExternalInput")
    out_t = nc.dram_tensor(
        "out", [n_blocks * P, D], mybir.dt.bfloat16, kind="ExternalOutput"
    )

    with tile.TileContext(nc) as tc:
        with (
            tc.tile_pool(name="static", bufs=1) as static_pool,
            tc.tile_pool(name="gat0", bufs=GAT_BUFS) as gp0,
            tc.tile_pool(name="gat1", bufs=GAT_BUFS) as gp1,
            tc.tile_pool(name="gat2", bufs=GAT_BUFS) as gp2,
            tc.tile_pool(name="gat3", bufs=GAT_BUFS) as gp3,
            tc.tile_pool(name="emb0", bufs=GAT_BUFS) as ep0,
            tc.tile_pool(name="emb1", bufs=GAT_BUFS) as ep1,
            tc.tile_pool(name="emb2", bufs=GAT_BUFS) as ep2,
            tc.tile_pool(name="emb3", bufs=GAT_BUFS) as ep3,
            tc.tile_pool(name="ohv0", bufs=GAT_BUFS) as op0,
            tc.tile_pool(name="ohv1", bufs=GAT_BUFS) as op1,
            tc.tile_pool(name="ohv2", bufs=GAT_BUFS) as op2,
            tc.tile_pool(name="ohv3", bufs=GAT_BUFS) as op3,
            tc.tile_pool(name="outp", bufs=4) as out_pool,
            tc.tile_pool(name="psum", bufs=8, space="PSUM") as psum_pool,
        ):
            idx_sb = static_pool.tile([P, n_chunks * (P // 16)], mybir.dt.int16)
            vals_sb = static_pool.tile([P, n_chunks], mybir.dt.float32)
            rrow_sb = static_pool.tile([P, n_chunks], mybir.dt.bfloat16)
            iota_sb = static_pool.tile([P, P, G_CHUNKS], mybir.dt.bfloat16)
            nc.sync.dma_start(out=idx_sb[:], in_=idx_t[:])
            nc.sync.dma_start(out=vals_sb[:], in_=vals_t[:])
            nc.sync.dma_start(out=rrow_sb[:], in_=rrow_t[:])
            nc.sync.dma_start(out=iota_sb[:], in_=iota_t[:])

            gat_pools = [gp0, gp1, gp2, gp3]
            emb_pools = [ep0, ep1, ep2, ep3]
            ohv_pools = [op0, op1, op2, op3]
            # per group: list of (emb_tile, ohv_tile) per gather batch
            btiles = [[] for _ in range(N_GROUPS)]
            next_instr = [0] * N_GROUPS
            qrr = [0]  # SWDGE queue round-robin
            prev_gather = [None]  # nosync chain so scheduled order == emission
            # order (tile's DMASW sem lanes rotate mod 8 in scheduled order;
            # each lane is locked to one SWDGE queue, so queue rotation must
            # stay aligned with lane rotation)

            def ensure(g, upto_gc):
                # emit batches for group g until group-chunks [0, upto_gc) covered
                while next_instr[g] * G_CHUNKS < upto_gc:
                    j = next_instr[g]
                    g0 = j * G_CHUNKS
                    n_i = int(min(G_CHUNKS, Tg[g] - g0))
                    slot0 = int(gbase[g] + g0)
                    gt = gat_pools[g].tile([P, G_CHUNKS, D], mybir.dt.float32)
                    r0 = g * group_size
                    r1 = min((g + 1) * group_size, n_nodes)
                    gi = nc.gpsimd.dma_gather(
                        gt[:, :n_i, :],
                        embeds_t[r0:r1, :],
                        idx_sb[:, slot0 * (P // 16) : (slot0 + n_i) * (P // 16)],
                        n_i * P,
                        n_i * P,
                        D,
                        queue_num=qrr[0],
                    )
                    if prev_gather[0] is not None:
                        from concourse.instruction_name_ordered_set import (
                            InstructionNameOrderedSet,
                        )

                        dep = InstructionNameOrderedSet()
                        dep.add(prev_gather[0])
                        gi.ins.add_nosync_dependencies_from(dep)
                    prev_gather[0] = gi.ins.name
                    qrr[0] = (qrr[0] + 1) % 4
                    emb = emb_pools[g].tile([P, G_CHUNKS, D], mybir.dt.bfloat16)
                    nc.vector.tensor_tensor(
                        out=emb[:, :n_i, :],
                        in0=gt[:, :n_i, :],
                        in1=vals_sb[:, slot0 : slot0 + n_i].to_broadcast(
                            [P, n_i, D]
                        ),
                        op=mybir.AluOpType.mult,
                    )
                    # one-hot in [P, r, c] layout: inner (chunk) dim is packed
                    # in all operands -> DVE 2x_1p mode applies
                    ohv = ohv_pools[g].tile([P, P, G_CHUNKS], mybir.dt.bfloat16)
                    nc.vector.tensor_tensor(
                        out=ohv[:, :, :n_i],
                        in0=rrow_sb[:, slot0 : slot0 + n_i]
                        .to_broadcast([P, n_i, P])
                        .transpose([0, 2, 1]),
                        in1=iota_sb[:, :, :n_i],
                        op=mybir.AluOpType.is_equal,
                    )
                    btiles[g].append((emb, ohv))
                    next_instr[g] += 1

            for b in range(n_blocks):
                tot_b = int(chunks_bg[b].sum())
                psum_tile = psum_pool.tile([P, D], dtype=mybir.dt.float32, space="PSUM")
                t = 0
                for g in range(N_GROUPS):
                    cbg = int(chunks_bg[b, g])
                    if cbg == 0:
                        continue
                    gc0 = int(off_bg[b, g] - gbase[g])
                    ensure(g, gc0 + cbg)
                    for c in range(cbg):
                        gc = gc0 + c
                        emb, ohv = btiles[g][gc // G_CHUNKS]
                        o = gc % G_CHUNKS
                        nc.tensor.matmul(
                            out=psum_tile[:],
                            lhsT=ohv[:, :, o],
                            rhs=emb[:, o, :],
                            start=(t == 0),
                            stop=(t == tot_b - 1),
                        )
                        t += 1
                o_sb = out_pool.tile([P, D], mybir.dt.bfloat16)
                nc.scalar.copy(out=o_sb[:], in_=psum_tile[:])
                nc.sync.dma_start(out=out_t[b * P : (b + 1) * P, :], in_=o_sb[:])
    nc.compile()
    return nc


def _kernel_impl(rows, cols, edge_vals, embeds, n_nodes, trace=False):
    import ml_dtypes

    from concourse.bass_utils import run_bass_kernel_spmd

    rows = np.asarray(rows).astype(np.int64)
    cs_all = np.asarray(cols).astype(np.int32)
    vs_all = np.asarray(edge_vals).astype(np.float32)
    embeds = np.ascontiguousarray(np.asarray(embeds), dtype=np.float32)

    npc = n_nodes // N_CORES
    assert npc * N_CORES == n_nodes
    n_blocks = (npc + P - 1) // P
    group_size = -(-n_nodes // N_GROUPS)
    assert group_size <= 32767

    core = rows // npc
    blk = (rows % npc) // P
    rrow = (rows % npc) % P
    grp = cs_all // group_size
    bkey = ((core * n_blocks + blk) * N_GROUPS + grp).astype(np.int64)
    order = np.argsort(bkey, kind="stable")
    bkey_s = bkey[order]
    cs_s = cs_all[order]
    vs_s = vs_all[order]
    rrow_s = rrow[order]

    n_seg = N_CORES * n_blocks * N_GROUPS
    cnt = np.bincount(bkey_s, minlength=n_seg).reshape(N_CORES, n_blocks, N_GROUPS)
    chunks_bg = -(-cnt.max(axis=0) // P)  # [n_blocks, N_GROUPS]
    forced = chunks_bg.sum(axis=1) == 0
    chunks_bg[forced, 0] = 1
    Tg, gbase, n_chunks, off_bg = _schedule(chunks_bg)

    # position of each edge inside its (core, block, group) segment
    seg_start = np.zeros(n_seg + 1, np.int64)
    np.cumsum(cnt.ravel(), out=seg_start[1:])
    pos_in_seg = np.arange(len(rows), dtype=np.int64) - seg_start[bkey_s]

    b_s = (bkey_s // N_GROUPS) % n_blocks
    g_s = bkey_s % N_GROUPS
    k_s = bkey_s // (n_blocks * N_GROUPS)
    slot_s = off_bg[b_s, g_s] + pos_in_seg // P  # global chunk slot
    part_s = pos_in_seg % P

    vals_p = np.zeros((N_CORES, n_chunks, P), np.float32)
    rrow_p = np.zeros((N_CORES, n_chunks, P), np.float32)
    tok_p = np.zeros((N_CORES, n_chunks, P), np.int16)
    vals_p[k_s, slot_s, part_s] = vs_s
    rrow_p[k_s, slot_s, part_s] = rrow_s
    tok_p[k_s, slot_s, part_s] = (cs_s - g_s * group_size).astype(np.int16)

    # device layouts
    vals_d = np.ascontiguousarray(vals_p.transpose(0, 2, 1))  # [8, P, n_chunks]
    rrow_d = np.ascontiguousarray(rrow_p.transpose(0, 2, 1)).astype(ml_dtypes.bfloat16)
    # idx wrap: token t at partition t%16, col t//16; replicate to 128 partitions
    idx_d = np.ascontiguousarray(
        np.tile(
            tok_p.reshape(N_CORES, n_chunks * P // 16, 16).transpose(0, 2, 1),
            (1, 8, 1),
        )
    )  # [8, 128, n_chunks*8]
    # iota[p, r, c] = r
    iota = np.ascontiguousarray(
        np.broadcast_to(
            np.arange(P, dtype=np.float32)[None, :, None], (P, P, G_CHUNKS)
        )
    ).astype(ml_dtypes.bfloat16)

    nc = _build_program(chunks_bg, n_nodes)
    in_maps = [
        {
            "embeds": embeds,
            "idx_p": idx_d[k],
            "vals_p": vals_d[k],
            "rrow_p": rrow_d[k],
            "iota": iota,
        }
        for k in range(N_CORES)
    ]
    global _LAST
    _LAST = (nc, in_maps)
    r = run_bass_kernel_spmd(nc, in_maps, list(range(N_CORES)), trace=trace)
    out = np.concatenate(
        [r.results[k]["out"][:npc] for k in range(N_CORES)], axis=0
    ).astype(np.float32)
    if trace:
        return out, r
    return out


_LAST = None


def kernel(rows, cols, edge_vals, embeds):
    return _kernel_impl(rows, cols, edge_vals, embeds, N_NODES)
